# revision 60
# baseline (speedup 1.0000x reference)
"""GAT BasicAttentionBlock kernel for 8x Trainium2 NeuronCores.

Strategy (output-shard, v2): each core owns 1250 of the 10000 selected
output rows (index0).  Only nodes reachable from those rows matter
(~1.2k targets + ~16k sources per core).  Per core:

  node table order: [window-grouped targets (1280 rows) | sources sorted
  by per-core edge multiplicity desc].  A static row boundary B1 (mult
  of 512) splits the table so rows < B1 carry ~2/3 of the edges.

  phase A  stream x columns of the table nodes (bf16), h = relu(x@w1.T)
           feature-major on PE, then per 128-node subtile
           proj|s_src = h@w2 node-major; copy cols 0:136 (bf16) of each
           PSUM tile to SBUF and DMA full 512-byte rows to the HBM table.
           Emission of loop-1 work (s_trg/skip/one-hot masks) is
           interleaved into phase A's engine slack.
  gathers  per 128-target window, edges sorted by source row: slots
           [0,KLO) hold only sources < B1 and are gathered as soon as
           the lo part of the table is written (overlaps phase A);
           slots [KLO,EC) gather after the full table.
  loop 2   per window: scores = lrelu(s_src+s_trg) on ACT (alpha=.2),
           e = exp, weighted = e*proj, segment-sum via one-hot matmuls
           accumulated in PSUM [sum e*proj | sum e]; out = att/den +
           skip, ELU; windows finish staggered as hi-gathers land.
  final    dma_gather the 1250 output rows from the 1280-target table.

No collectives: cores are fully independent.  The softmax global max
subtraction cancels in att = exp/sum(exp) and is dropped.
"""

import os
import sys

for _p in ("/opt/trn_rl_repo",):
    if os.path.isdir(_p) and _p not in sys.path:
        sys.path.insert(0, _p)

import numpy as np
import ml_dtypes

# problem constants (hardcoded per contract)
N = 50000
E = 800000
K = 10000
IN = 256
H = 128
NH = 8
HD = 16
OC = NH * HD  # 128
CORES = 8
KC = K // CORES          # 1250 output rows per core
P = 128
W = 10                   # target windows of 128 -> 1280 target slots
TP = W * P               # padded target count per core
EPS = 1e-16

BF16 = ml_dtypes.bfloat16

LOFRAC = float(os.environ.get("KLOFRAC", "0.55"))


# ----------------------------------------------------------------------------
# host-side sharding / planning
# ----------------------------------------------------------------------------

def _wrap16(vals, reps=8):
    """int16 index layout for dma_gather: idx i at [i%16, i//16], the 16-row
    block replicated `reps` times down the partition axis."""
    L = vals.shape[0]
    assert L % 16 == 0
    w = vals.reshape(L // 16, 16).T.astype(np.int16)
    return np.tile(w, (reps, 1))


def _binpack(deg):
    """Assign targets (by degree desc) to W windows (<=128 each), balancing
    total degree.  Returns row index (w*128 + pos) per target."""
    U = len(deg)
    order = np.argsort(-deg, kind="stable")
    wdeg = np.zeros(W)
    wcnt = np.zeros(W, np.int64)
    row = np.zeros(U, np.int64)
    for u in order:
        cand = np.nonzero(wcnt < P)[0]
        wsel = cand[np.argmin(wdeg[cand])]
        row[u] = wsel * P + wcnt[wsel]
        wcnt[wsel] += 1
        wdeg[wsel] += deg[u]
    return row


def plan(x, adj0, index0):
    src_all = np.asarray(adj0[0], dtype=np.int64)
    trg_all = np.asarray(adj0[1], dtype=np.int64)
    idx0 = np.asarray(index0, dtype=np.int64)
    x = np.asarray(x, dtype=np.float32)

    pre = []
    npad_req = 512
    for c in range(CORES):
        ks = idx0[c * KC:(c + 1) * KC]
        tgt_u, inv_k = np.unique(ks, return_inverse=True)
        U_t = len(tgt_u)
        assert U_t <= TP
        lut = np.full(N, -1, np.int64)
        lut[tgt_u] = np.arange(U_t)
        tloc_all = lut[trg_all]
        sel = np.nonzero(tloc_all >= 0)[0]
        e_src = src_all[sel]
        e_tu = tloc_all[sel]
        deg = np.bincount(e_tu, minlength=U_t)
        trow = _binpack(deg)                       # tgt_u idx -> table row

        # source rows: targets keep their rows; extras sorted by edge count
        nrow = np.full(N, -1, np.int64)
        nrow[tgt_u] = trow
        is_extra = nrow[e_src] < 0
        ex_ids, ex_cnt_inv = np.unique(e_src[is_extra], return_inverse=True)
        ex_cnt = np.bincount(ex_cnt_inv)
        ex_order = np.argsort(-ex_cnt, kind="stable")
        extras = ex_ids[ex_order]
        nrow[extras] = TP + np.arange(len(extras))
        U_n = TP + len(extras)
        npad_req = max(npad_req, U_n)

        e_srow = nrow[e_src]                       # source table row per edge
        e_trow = trow[e_tu]                        # target table row per edge
        # node id per table row (for xT); pad rows -> x of node 0 (harmless)
        nodes = np.zeros(U_n, np.int64)
        nodes[trow] = tgt_u
        nodes[TP:] = extras
        pre.append((trow, inv_k, e_srow, e_trow, nodes, U_n))

    # direct class: KD slots/window of mult-1 non-target sources whose
    # proj is written straight into SBUF Ghi during phase A (no table row,
    # no gather).  Everything else is gathered from tabLo.
    KD = 7
    kg_req = 1
    packed = []
    for c in range(CORES):
        trow, inv_k, e_srow_unused, e_trow, nodes_unused, U_n = pre[c]
        # recompute from raw edge lists kept in pre
        packed.append(None)

    per_core = []
    NL_req = 512
    info = []
    for c in range(CORES):
        ks = idx0[c * KC:(c + 1) * KC]
        tgt_u, inv_k = np.unique(ks, return_inverse=True)
        lut = np.full(N, -1, np.int64)
        lut[tgt_u] = np.arange(len(tgt_u))
        tloc_all = lut[trg_all]
        sel = np.nonzero(tloc_all >= 0)[0]
        e_src = src_all[sel]
        e_tu = tloc_all[sel]
        deg = np.bincount(e_tu, minlength=len(tgt_u))
        trow = _binpack(deg)
        e_trow = trow[e_tu]
        e_win = e_trow >> 7

        is_tgt = np.zeros(N, bool)
        is_tgt[tgt_u] = True
        cnt = np.bincount(e_src, minlength=N)
        m1 = (cnt[e_src] == 1) & (~is_tgt[e_src])

        # per window: first KD*128 mult-1 edges are direct
        direct = np.zeros(len(e_src), bool)
        for w in range(W):
            idx = np.nonzero(m1 & (e_win == w))[0]
            assert len(idx) >= KD * P, (c, w, len(idx))
            direct[idx[:KD * P]] = True
        gcnt = np.bincount(e_win[~direct], minlength=W)
        kg_req = max(kg_req, int(np.ceil(gcnt.max() / P)))

        # tabLo rows: targets first, then unique gathered sources
        g_src = e_src[~direct]
        nrow = np.full(N, -1, np.int64)
        nrow[tgt_u] = trow
        extras = np.setdiff1d(np.unique(g_src), tgt_u)
        nrow[extras] = TP + np.arange(len(extras))
        NL_req = max(NL_req, TP + len(extras))
        info.append((tgt_u, inv_k, trow, e_src, e_trow, e_win, direct, nrow))

    KLO = kg_req
    KHI = KD
    EC = KLO + KHI
    cap = EC * P
    ND = W * KD * P                    # direct nodes (8960)
    NDP = ((ND + 1023) // 1024) * 1024  # padded to chunk mult (9216)
    NL = ((NL_req + 1023) // 1024) * 1024
    NPAD = NDP + NL
    B1 = NL  # tabLo tiles first; direct tiles start here

    for c in range(CORES):
        tgt_u, inv_k, trow, e_src, e_trow, e_win, direct, nrow = info[c]
        x_nodes = np.zeros(NPAD, np.int64)

        etcol = np.full((P, W * EC), -1.0, np.float32)
        esrc_g = np.zeros((W, KLO * P), np.int64)

        # direct edges: block b = w*KD + jd -> xTi position NL + b*128 + p
        for w in range(W):
            idx = np.nonzero(direct & (e_win == w))[0]
            for jd in range(KD):
                blk = idx[jd * P:(jd + 1) * P]
                b = w * KD + jd
                x_nodes[NL + b * P:NL + (b + 1) * P] = e_src[blk]
                etcol[:, w * EC + KLO + jd] = \
                    (e_trow[blk] - w * P).astype(np.float32)

        # gathered edges per window, packed into slots [0, KLO*P)
        for w in range(W):
            idx = np.nonzero((~direct) & (e_win == w))[0]
            ng = len(idx)
            esrc_g[w, :ng] = nrow[e_src[idx]]
            ec_ = np.full(KLO * P, -1.0, np.float32)
            ec_[:ng] = (e_trow[idx] - w * P).astype(np.float32)
            etcol[:, w * EC:w * EC + KLO] = \
                ec_.reshape(KLO, P).T

        # tabLo node ids at xTi positions [0, NL)
        rows_used = np.nonzero(nrow >= 0)[0]
        x_nodes[nrow[rows_used]] = rows_used

        # etrow for partition-broadcast (slot-major per window)
        etrow_b = np.empty((1, W * cap), BF16)
        for w in range(W):
            etrow_b[0, w * cap:(w + 1) * cap] = \
                etcol[:, w * EC:(w + 1) * EC].T.reshape(-1).astype(BF16)

        eidx_lo = np.concatenate(
            [_wrap16(esrc_g[w]) for w in range(W)], axis=1)

        kvals = np.zeros(TP, np.int64)
        kvals[:KC] = trow[inv_k]
        kidx = _wrap16(kvals)

        xT = np.zeros((IN, NPAD), BF16)
        xT[:, :] = x[x_nodes].T
        CW = 1024
        assert NPAD % CW == 0
        xTi = np.empty((P, 2 * NPAD), BF16)
        for i in range(NPAD // CW):
            xTi[:, 2 * i * CW:2 * i * CW + CW] = xT[0:P, i * CW:(i + 1) * CW]
            xTi[:, 2 * i * CW + CW:2 * (i + 1) * CW] = \
                xT[P:IN, i * CW:(i + 1) * CW]

        iblob = np.concatenate([eidx_lo, kidx], axis=1)
        per_core.append(dict(xTi=xTi, iblob=iblob,
                             etcol=np.ascontiguousarray(etcol),
                             etrow=etrow_b))
    return per_core, NPAD, EC, KLO, B1


def make_weights(w_in, b_in, w_proj, a_src, a_trg, w_skip):
    w_in = np.asarray(w_in, np.float32)
    b_in = np.asarray(b_in, np.float32)
    w_proj = np.asarray(w_proj, np.float32)
    a_src = np.asarray(a_src, np.float32).reshape(NH, HD)
    a_trg = np.asarray(a_trg, np.float32).reshape(NH, HD)
    w_skip = np.asarray(w_skip, np.float32)

    w1T = np.ascontiguousarray(w_in.T).astype(BF16)        # [256,128]
    b1 = b_in.reshape(H, 1).astype(np.float32)
    # B_src[h, a] = sum_d w_proj[a*16+d, h] * a_src[a, d]
    wp3 = w_proj.reshape(NH, HD, H)
    B_src = np.einsum("adh,ad->ha", wp3, a_src).astype(np.float32)  # [128,8]
    B_trg = np.einsum("adh,ad->ha", wp3, a_trg).astype(BF16)
    w2 = np.zeros((H, 256), np.float32)  # cast to bf16 below
    w2[:, :OC] = w_proj.T
    w2[:, OC:OC + NH] = B_src
    wskT = np.ascontiguousarray(w_skip.T).astype(BF16)     # [128,128]
    iota4 = np.arange(P, dtype=BF16)[None, :].repeat(P, axis=0)
    iota_c = np.arange(P, dtype=np.float32).reshape(P, 1)
    bfblob = np.concatenate(
        [np.ascontiguousarray(w1T[0:P]), np.ascontiguousarray(w1T[P:IN]),
         w2.astype(BF16), wskT, B_trg, iota4], axis=1)  # [128, 776]
    return dict(bfblob=bfblob, b1=b1, iota_c=iota_c)


# ----------------------------------------------------------------------------
# bass kernel
# ----------------------------------------------------------------------------

_BUILD_CACHE = {}


def build(NPAD, EC, KLO, B1):
    key = (NPAD, EC, KLO, B1)
    if key in _BUILD_CACHE:
        return _BUILD_CACHE[key]

    import concourse.bacc as bacc
    import concourse.mybir as mybir
    import concourse.tile as tile

    dt = mybir.dt
    F32 = dt.float32
    F32R = dt.float32r
    I16 = dt.int16
    BF = dt.bfloat16
    AF = mybir.ActivationFunctionType
    OP = mybir.AluOpType

    NT = NPAD // 512
    cap = EC * P
    KHI = EC - KLO

    nc = bacc.Bacc("TRN2", target_bir_lowering=False,
                   num_swdge_queues=4)

    with tile.TileContext(nc) as tc:
        with tc.tile_pool(name="dram", bufs=1, space="DRAM") as dram:
            def din(name, shape, dtp):
                return dram.tile(shape, dtp, kind="ExternalInput", name=name,
                                 uniquify=False)

            NBF = H + H + 256 + OC + NH + P  # 776
            NI16 = W * KLO * 8 + TP // 16
            xTi = din("xTi", [P, 2 * NPAD], BF)
            bfblob = din("bfblob", [P, NBF], BF)
            fblob = din("fblob", [P, 2 + W * EC], F32)
            iblob = din("iblob", [P, NI16], I16)
            etrow = din("etrow", [1, W * cap], BF)

            tabLo = dram.tile([NPAD - B1, 256], BF, kind="Internal",
                              name="tabLo", uniquify=False)
            outT = dram.tile([TP, OC], BF, kind="Internal", name="outT",
                             uniquify=False)
            out = dram.tile([TP, OC], BF, kind="ExternalOutput", name="out",
                            uniquify=False)

        with tc.tile_pool(name="pers", bufs=1) as pers:
            bfb = pers.tile([P, NBF], BF)
            fb = pers.tile([P, 2 + W * EC], F32)
            ib = pers.tile([P, NI16], I16)
            hfmt = pers.tile([H, TP], BF)         # targets' h, feature-major
            strg = pers.tile([P, W * NH], BF)     # per-window s_trg  [t, 8]
            skips = pers.tile([P, W, OC], BF)     # per-window skip   [t, oc]
            st_sb = pers.tile([P, W, EC, NH], BF)   # s_trg per edge slot
            Mw = pers.tile([P, W * cap], BF)      # edge->target one-hot
            Ghi = pers.tile([P, W, EC - KLO, 256], BF)  # hi-gathered rows
            etws = pers.tile([1, W * cap], BF)

            nc.sync.dma_start(etws[:], etrow[:])
            nc.sync.dma_start(fb[:], fblob[:])
            nc.sync.dma_start(bfb[:], bfblob[:])
            nc.sync.dma_start(ib[:], iblob[:])

            w1a = bfb[:, 0:H]
            w1b = bfb[:, H:2 * H]
            w2s = bfb[:, 2 * H:2 * H + 256]
            wsks = bfb[:, 2 * H + 256:2 * H + 256 + OC]
            btrgs = bfb[:, 2 * H + 256 + OC:2 * H + 256 + OC + NH]
            iota4s = bfb[:, 2 * H + 256 + OC + NH:NBF]
            b1s = fb[:, 0:1]
            iotac = fb[:, 1:2]
            etcols = fb[:, 2:2 + W * EC]
            eloidx = ib[:, 0:W * KLO * 8]
            kidxs = ib[:, W * KLO * 8:NI16]

            CH = 2  # 512-node tiles per xT load chunk
            with tc.tile_pool(name="pa", bufs=2) as pa, \
                 tc.tile_pool(name="pax", bufs=3) as pax, \
                 tc.tile_pool(name="pbc", bufs=2) as pbc, \
                 tc.tile_pool(name="pmtw", bufs=1) as pmtw, \
                 tc.tile_pool(name="pghi", bufs=3) as pghi, \
                 tc.tile_pool(name="pe2", bufs=2) as pe2, \
                 tc.tile_pool(name="psa", bufs=2, space="PSUM") as psa, \
                 tc.tile_pool(name="psb", bufs=2, space="PSUM") as psb, \
                 tc.tile_pool(name="psc", bufs=1, space="PSUM") as psc, \
                 tc.tile_pool(name="psd", bufs=1, space="PSUM") as psd, \
                 tc.tile_pool(name="pse", bufs=2, space="PSUM") as pse:

                # ---- partition-broadcast of per-slot target cols (Pool) ----
                pbcs = []
                for w in range(W):
                    pbcw = pbc.tile([P, cap], BF, tag="pbcw")
                    nc.gpsimd.partition_broadcast(
                        pbcw[:], etws[0:1, w * cap:(w + 1) * cap])
                    pbcs.append(pbcw)

                # deferred emissions interleaved into phase A slack
                mtws = {}

                def emit_mtw(w):
                    Mtw = pmtw.tile([P, cap], BF, tag="Mtw")
                    nc.vector.tensor_scalar(Mtw[:], pbcs[w][:], iotac[:], None,
                                            OP.is_equal)
                    mtws[w] = Mtw

                def emit_loop1(w):
                    # s_trg / skip for the window targets
                    stp = psd.tile([P, OC], F32, tag="misc")
                    nc.tensor.matmul(stp[:, 0:NH],
                                     lhsT=hfmt[:, w * P:(w + 1) * P],
                                     rhs=btrgs[:], start=True, stop=True)
                    nc.vector.tensor_copy(strg[:, w * NH:(w + 1) * NH],
                                            stp[:, 0:NH])
                    skp = psd.tile([P, OC], F32, tag="misc")
                    nc.tensor.matmul(skp[:], lhsT=hfmt[:, w * P:(w + 1) * P],
                                     rhs=wsks[:], start=True, stop=True)
                    nc.vector.tensor_copy(skips[:, w], skp[:])
                    # s_trg edge-slot expansion via the col-major one-hot
                    Mtw = mtws.pop(w)
                    stps = psc.tile([P, EC, NH], F32, tag="stps")
                    for j in range(EC):
                        nc.tensor.matmul(
                            stps[:, j, :], lhsT=Mtw[:, j * P:(j + 1) * P],
                            rhs=strg[:, w * NH:(w + 1) * NH],
                            start=True, stop=True)
                    nc.vector.tensor_copy(st_sb[:, w], stps[:])

                def emit_mw(w, j, eng=None):
                    col = w * EC + j
                    (eng or nc.vector).tensor_scalar(
                        Mw[:, col * P:(col + 1) * P], iota4s[:, 0:P],
                        etcols[:, col:col + 1], None, OP.is_equal)

                # schedule: loop1(w) at tile 2+w; Mw slots spread over tiles

                # ------- phase A (tabLo tiles first, then direct) -------
                LT = B1 // 512  # tabLo tiles
                for ci in range(NT // CH):
                    t0 = ci * CH
                    wdc = CH * 512
                    xc = pax.tile([P, 2 * wdc], BF, tag="xc")
                    nc.sync.dma_start(xc[:], xTi[:, 2 * ci * wdc:
                                                 2 * (ci + 1) * wdc])
                    stg = pa.tile([P, 2, 4, 256], BF, tag="stg")
                    for t in range(t0, t0 + CH):
                        o = (t - t0) * 512
                        hps = psa.tile([P, 512], F32, tag="hps")
                        nc.tensor.matmul(hps[:], lhsT=w1a[:],
                                         rhs=xc[:, o:o + 512],
                                         start=True, stop=False)
                        nc.tensor.matmul(hps[:], lhsT=w1b[:],
                                         rhs=xc[:, wdc + o:wdc + o + 512],
                                         start=False, stop=True)
                        hsb = pa.tile([P, 512], BF, tag="hsb")
                        nc.scalar.activation(hsb[:], hps[:], AF.Relu,
                                             bias=b1s[:])
                        if t * 512 < TP:
                            w0 = t * 512
                            w1_ = min(TP, w0 + 512)
                            nc.scalar.activation(hfmt[:, w0:w1_],
                                                 hps[:, 0:(w1_ - w0)], AF.Relu,
                                                 bias=b1s[:])
                        for half in range(2):
                            p2 = psb.tile([P, 2, 256], F32, tag="p2")
                            for jj in range(2):
                                j = half * 2 + jj
                                nc.tensor.matmul(
                                    p2[:, jj, :],
                                    lhsT=hsb[:, j * P:(j + 1) * P],
                                    rhs=w2s[:], start=True, stop=True)
                            if t >= LT:
                                # direct class: copy straight into Ghi; the
                                # host packed these nodes in edge-slot order
                                b0 = (t - LT) * 4 + half * 2
                                w_, jd = b0 // KHI, b0 % KHI
                                if jd != KHI - 1 and b0 + 1 < W * KHI:
                                    # both blocks same window, adjacent slots
                                    dst = Ghi[:, w_, jd:jd + 2, 0:OC + NH]
                                    s2 = p2[:, :, 0:OC + NH]
                                    if half == 0:
                                        nc.scalar.activation(dst, s2, AF.Copy)
                                    else:
                                        nc.vector.tensor_copy(dst, s2)
                                else:
                                    for sub in range(2):
                                        b = b0 + sub
                                        if b >= W * KHI:
                                            continue
                                        wx, jx = b // KHI, b % KHI
                                        dst = Ghi[:, wx, jx, 0:OC + NH]
                                        s1 = p2[:, sub, 0:OC + NH]
                                        if half == 0:
                                            nc.scalar.activation(dst, s1,
                                                                 AF.Copy)
                                        else:
                                            nc.vector.tensor_copy(dst, s1)
                            else:
                                sgh = stg[:, t - t0, half * 2:half * 2 + 2, :]
                                if half == 0:
                                    nc.scalar.activation(sgh[:, :, 0:OC + NH],
                                                         p2[:, :, 0:OC + NH],
                                                         AF.Copy)
                                else:
                                    nc.vector.tensor_copy(sgh[:, :, 0:OC + NH],
                                                          p2[:, :, 0:OC + NH])
                    if t0 < LT:
                        rr = t0 * 512
                        nc.sync.dma_start(
                            tabLo[rr:rr + CH * 512, :].rearrange(
                                "(i j p) f -> p i j f", p=P, i=CH), stg[:])
                    # interleaved loop-1 / mask emissions (by position)
                    for pi in (ci * CH, ci * CH + 1):
                        if 1 <= pi <= 2 * W and pi % 2 == 1:
                            emit_mtw((pi - 1) // 2)
                        if 2 <= pi <= 2 * W + 1 and pi % 2 == 0:
                            emit_loop1((pi - 2) // 2)

                # edge->target one-hot masks: fills the DVE gap between
                # phase A and the window chains; every 4th slot on Pool
                for w_ in range(W):
                    for j_ in range(EC):
                        emit_mw(w_, j_)

                # ---------------- gathers ----------------
                # direct-class rows are already in Ghi (phase A); only the
                # gathered class reads tabLo, in window pairs
                glos = []
                for w in range(W):
                    G = pghi.tile([P, KLO, 256], BF, tag="G")
                    nc.gpsimd.dma_gather(
                        G[:], tabLo[:],
                        eloidx[:, w * KLO * 8:(w + 1) * KLO * 8],
                        KLO * P, KLO * P, 256, single_packet=False,
                        queue_num=1 + w % 3)
                    glos.append(G)

                # ---------------- loop 2: per-window edge pipeline ----------
                def finalize(w, segp):
                    den = pe2.tile([P, NH], F32, tag="den")
                    nc.vector.tensor_scalar_add(den[:], segp[:, OC:OC + NH],
                                                EPS)
                    rec = pe2.tile([P, NH], F32, tag="rec")
                    nc.vector.reciprocal(rec[:], den[:])
                    z = pe2.tile([P, OC], F32, tag="z")
                    recb = rec[:].broadcast_to([P, NH, HD])
                    nc.vector.tensor_tensor(
                        z[:].rearrange("p (a d) -> p a d", d=HD),
                        segp[:, 0:OC].rearrange("p (a d) -> p a d", d=HD),
                        recb, OP.mult)
                    nc.gpsimd.tensor_add(z[:], z[:], skips[:, w])
                    # elu: (max(z,0)-1) + exp(min(z,0))
                    am = pe2.tile([P, OC], BF, tag="am")
                    nc.gpsimd.tensor_scalar(am[:], z[:], 0.0, -1.0, OP.max,
                                            OP.add)
                    bm = pe2.tile([P, OC], BF, tag="bm")
                    nc.gpsimd.tensor_scalar(bm[:], z[:], 0.0, None, OP.min)
                    eb = pe2.tile([P, OC], BF, tag="eb")
                    nc.scalar.activation(eb[:], bm[:], AF.Exp)
                    nc.vector.tensor_add(am[:], am[:], eb[:])
                    nc.sync.dma_start(outT[w * P:(w + 1) * P, :], am[:])

                # software-pipelined window stages: each engine's
                # in-order queue interleaves adjacent windows
                st1 = {}   # w -> (sc-dependent) emax tile
                st2 = {}   # w -> Wv tile
                st3 = {}   # w -> segp psum tile

                def stage1(w):
                    G = glos[w]
                    sc = pe2.tile([P, EC, NH], F32, tag="sc")
                    nc.vector.tensor_tensor(sc[:, 0:KLO], st_sb[:, w, 0:KLO],
                                            G[:, :, OC:OC + NH], OP.add)
                    nc.vector.tensor_tensor(sc[:, KLO:EC],
                                            st_sb[:, w, KLO:EC],
                                            Ghi[:, w, :, OC:OC + NH], OP.add)
                    # exp(lrelu(s)) = exp(max(s, 0.2*s)): max first, 1 exp
                    e2 = pe2.tile([P, EC, NH], BF, tag="e2")
                    nc.vector.tensor_scalar(e2[:], sc[:], 0.2, None, OP.mult)
                    nc.vector.tensor_max(e2[:], e2[:], sc[:])
                    e1 = pe2.tile([P, EC, NH], BF, tag="e1")
                    nc.scalar.activation(e1[:], e2[:], AF.Exp)
                    st1[w] = e1

                def stage2(w):
                    G = glos[w]
                    emax = st1.pop(w)
                    Wv = pe2.tile([P, EC, 136], BF, tag="Wv")
                    nc.vector.tensor_copy(Wv[:, :, OC:OC + NH], emax[:])
                    emb = emax[:].broadcast_to([P, EC, NH, HD])
                    SPL = KLO + 3
                    pool = pmtw if w % 2 == 0 else pbc
                    tag = "Mtw" if w % 2 == 0 else "pbcw"
                    eex = pool.tile([P, cap], BF, tag=tag)
                    ex3 = eex[:].rearrange("p (j f) -> p j f", f=P)
                    nc.scalar.activation(
                        ex3[:, 0:SPL].rearrange("p j (a d) -> p j a d", d=HD),
                        emb[:, 0:SPL], AF.Copy)
                    nc.vector.tensor_tensor(Wv[:, 0:KLO, 0:OC],
                                            G[:, :, 0:OC],
                                            ex3[:, 0:KLO], OP.mult)
                    nc.vector.tensor_tensor(Wv[:, KLO:SPL, 0:OC],
                                            Ghi[:, w, 0:SPL - KLO, 0:OC],
                                            ex3[:, KLO:SPL], OP.mult)
                    wvh = Wv[:, SPL:EC, 0:OC].rearrange(
                        "p j (a d) -> p j a d", d=HD)
                    ghi4 = Ghi[:, w, SPL - KLO:KHI, 0:OC].rearrange(
                        "p j (a d) -> p j a d", d=HD)
                    nc.vector.tensor_tensor(wvh, ghi4, emb[:, SPL:EC],
                                            OP.mult)
                    st2[w] = Wv

                def stage3(w):
                    Wv = st2.pop(w)
                    segp = pse.tile([P, 136], F32, tag="segp")
                    for j in range(EC):
                        nc.tensor.matmul(segp[:],
                                         lhsT=Mw[:, (w * EC + j) * P:
                                                 (w * EC + j + 1) * P],
                                         rhs=Wv[:, j, :], start=(j == 0),
                                         stop=(j == EC - 1))
                    st3[w] = segp

                def finalize(w):
                    segp = st3.pop(w)
                    den = pe2.tile([P, NH], F32, tag="den")
                    nc.vector.tensor_scalar_add(den[:], segp[:, OC:OC + NH],
                                                EPS)
                    rec = pe2.tile([P, NH], F32, tag="rec")
                    nc.vector.reciprocal(rec[:], den[:])
                    z = pe2.tile([P, OC], F32, tag="z")
                    recb = rec[:].broadcast_to([P, NH, HD])
                    nc.vector.tensor_tensor(
                        z[:].rearrange("p (a d) -> p a d", d=HD),
                        segp[:, 0:OC].rearrange("p (a d) -> p a d", d=HD),
                        recb, OP.mult)
                    nc.gpsimd.tensor_add(z[:], z[:], skips[:, w])
                    # elu: (max(z,0)-1) + exp(min(z,0))
                    am = pe2.tile([P, OC], BF, tag="am")
                    nc.gpsimd.tensor_scalar(am[:], z[:], 0.0, -1.0, OP.max,
                                            OP.add)
                    bm = pe2.tile([P, OC], BF, tag="bm")
                    nc.gpsimd.tensor_scalar(bm[:], z[:], 0.0, None, OP.min)
                    eb = pe2.tile([P, OC], BF, tag="eb")
                    nc.scalar.activation(eb[:], bm[:], AF.Exp)
                    nc.vector.tensor_add(am[:], am[:], eb[:])
                    nc.sync.dma_start(outT[w * P:(w + 1) * P, :], am[:])

                for w in range(W + 3):
                    if w < W:
                        stage1(w)
                    if 1 <= w <= W:
                        stage2(w - 1)
                    if 2 <= w <= W + 1:
                        stage3(w - 2)
                    if 3 <= w:
                        finalize(w - 3)

                # final k-row gather (reuses a pghi slot)
                kob = pghi.tile([P, KLO, 256], BF, tag="G")
                ko = kob[:].rearrange("p k f -> p (k f)")[:, 0:TP // P * OC]
                ko3 = ko.rearrange("p (j f) -> p j f", f=OC)
                nc.gpsimd.dma_gather(ko3, outT[:], kidxs[:], TP, TP, OC,
                                     single_packet=False)
                nc.sync.dma_start(
                    out[:].rearrange("(j p) f -> p j f", p=P), ko3)

    nc.compile()
    _BUILD_CACHE[key] = nc
    return nc


# ----------------------------------------------------------------------------
# entry point
# ----------------------------------------------------------------------------

def kernel(x, adj0, index0, w_in, b_in, w_proj, a_src, a_trg, w_skip):
    from concourse.bass_utils import run_bass_kernel_spmd

    per_core, NPAD, EC, KLO, B1 = plan(x, adj0, index0)
    wts = make_weights(w_in, b_in, w_proj, a_src, a_trg, w_skip)
    nc = build(NPAD, EC, KLO, B1)

    in_maps = []
    for c in range(CORES):
        pc = per_core[c]
        fblob = np.concatenate(
            [wts["b1"], wts["iota_c"], pc["etcol"]], axis=1).astype(np.float32)
        in_maps.append(dict(bfblob=wts["bfblob"], fblob=fblob,
                            xTi=pc["xTi"], iblob=pc["iblob"],
                            etrow=pc["etrow"]))

    res = run_bass_kernel_spmd(nc, in_maps, core_ids=list(range(CORES)))
    outs = [r["out"][:KC] for r in res.results]
    return np.concatenate(outs, axis=0).astype(np.float32)


# revision 61
# speedup vs baseline: 1.0174x; 1.0174x over previous
"""GAT BasicAttentionBlock kernel for 8x Trainium2 NeuronCores.

Strategy (output-shard, v2): each core owns 1250 of the 10000 selected
output rows (index0).  Only nodes reachable from those rows matter
(~1.2k targets + ~16k sources per core).  Per core:

  node table order: [window-grouped targets (1280 rows) | sources sorted
  by per-core edge multiplicity desc].  A static row boundary B1 (mult
  of 512) splits the table so rows < B1 carry ~2/3 of the edges.

  phase A  stream x columns of the table nodes (bf16), h = relu(x@w1.T)
           feature-major on PE, then per 128-node subtile
           proj|s_src = h@w2 node-major; copy cols 0:136 (bf16) of each
           PSUM tile to SBUF and DMA full 512-byte rows to the HBM table.
           Emission of loop-1 work (s_trg/skip/one-hot masks) is
           interleaved into phase A's engine slack.
  gathers  per 128-target window, edges sorted by source row: slots
           [0,KLO) hold only sources < B1 and are gathered as soon as
           the lo part of the table is written (overlaps phase A);
           slots [KLO,EC) gather after the full table.
  loop 2   per window: scores = lrelu(s_src+s_trg) on ACT (alpha=.2),
           e = exp, weighted = e*proj, segment-sum via one-hot matmuls
           accumulated in PSUM [sum e*proj | sum e]; out = att/den +
           skip, ELU; windows finish staggered as hi-gathers land.
  final    dma_gather the 1250 output rows from the 1280-target table.

No collectives: cores are fully independent.  The softmax global max
subtraction cancels in att = exp/sum(exp) and is dropped.
"""

import os
import sys

for _p in ("/opt/trn_rl_repo",):
    if os.path.isdir(_p) and _p not in sys.path:
        sys.path.insert(0, _p)

import numpy as np
import ml_dtypes

# problem constants (hardcoded per contract)
N = 50000
E = 800000
K = 10000
IN = 256
H = 128
NH = 8
HD = 16
OC = NH * HD  # 128
CORES = 8
KC = K // CORES          # 1250 output rows per core
P = 128
W = 10                   # target windows of 128 -> 1280 target slots
TP = W * P               # padded target count per core
EPS = 1e-16

BF16 = ml_dtypes.bfloat16

LOFRAC = float(os.environ.get("KLOFRAC", "0.55"))


# ----------------------------------------------------------------------------
# host-side sharding / planning
# ----------------------------------------------------------------------------

def _wrap16(vals, reps=8):
    """int16 index layout for dma_gather: idx i at [i%16, i//16], the 16-row
    block replicated `reps` times down the partition axis."""
    L = vals.shape[0]
    assert L % 16 == 0
    w = vals.reshape(L // 16, 16).T.astype(np.int16)
    return np.tile(w, (reps, 1))


def _binpack(deg):
    """Assign targets (by degree desc) to W windows (<=128 each), balancing
    total degree.  Returns row index (w*128 + pos) per target."""
    U = len(deg)
    order = np.argsort(-deg, kind="stable")
    wdeg = np.zeros(W)
    wcnt = np.zeros(W, np.int64)
    row = np.zeros(U, np.int64)
    for u in order:
        cand = np.nonzero(wcnt < P)[0]
        wsel = cand[np.argmin(wdeg[cand])]
        row[u] = wsel * P + wcnt[wsel]
        wcnt[wsel] += 1
        wdeg[wsel] += deg[u]
    return row


def plan(x, adj0, index0):
    src_all = np.asarray(adj0[0], dtype=np.int64)
    trg_all = np.asarray(adj0[1], dtype=np.int64)
    idx0 = np.asarray(index0, dtype=np.int64)
    x = np.asarray(x, dtype=np.float32)

    pre = []
    npad_req = 512
    for c in range(CORES):
        ks = idx0[c * KC:(c + 1) * KC]
        tgt_u, inv_k = np.unique(ks, return_inverse=True)
        U_t = len(tgt_u)
        assert U_t <= TP
        lut = np.full(N, -1, np.int64)
        lut[tgt_u] = np.arange(U_t)
        tloc_all = lut[trg_all]
        sel = np.nonzero(tloc_all >= 0)[0]
        e_src = src_all[sel]
        e_tu = tloc_all[sel]
        deg = np.bincount(e_tu, minlength=U_t)
        trow = _binpack(deg)                       # tgt_u idx -> table row

        # source rows: targets keep their rows; extras sorted by edge count
        nrow = np.full(N, -1, np.int64)
        nrow[tgt_u] = trow
        is_extra = nrow[e_src] < 0
        ex_ids, ex_cnt_inv = np.unique(e_src[is_extra], return_inverse=True)
        ex_cnt = np.bincount(ex_cnt_inv)
        ex_order = np.argsort(-ex_cnt, kind="stable")
        extras = ex_ids[ex_order]
        nrow[extras] = TP + np.arange(len(extras))
        U_n = TP + len(extras)
        npad_req = max(npad_req, U_n)

        e_srow = nrow[e_src]                       # source table row per edge
        e_trow = trow[e_tu]                        # target table row per edge
        # node id per table row (for xT); pad rows -> x of node 0 (harmless)
        nodes = np.zeros(U_n, np.int64)
        nodes[trow] = tgt_u
        nodes[TP:] = extras
        pre.append((trow, inv_k, e_srow, e_trow, nodes, U_n))

    # direct class: KD slots/window of mult-1 non-target sources whose
    # proj is written straight into SBUF Ghi during phase A (no table row,
    # no gather).  Everything else is gathered from tabLo.
    KD = 7
    kg_req = 1
    packed = []
    for c in range(CORES):
        trow, inv_k, e_srow_unused, e_trow, nodes_unused, U_n = pre[c]
        # recompute from raw edge lists kept in pre
        packed.append(None)

    per_core = []
    NL_req = 512
    info = []
    for c in range(CORES):
        ks = idx0[c * KC:(c + 1) * KC]
        tgt_u, inv_k = np.unique(ks, return_inverse=True)
        lut = np.full(N, -1, np.int64)
        lut[tgt_u] = np.arange(len(tgt_u))
        tloc_all = lut[trg_all]
        sel = np.nonzero(tloc_all >= 0)[0]
        e_src = src_all[sel]
        e_tu = tloc_all[sel]
        deg = np.bincount(e_tu, minlength=len(tgt_u))
        trow = _binpack(deg)
        e_trow = trow[e_tu]
        e_win = e_trow >> 7

        is_tgt = np.zeros(N, bool)
        is_tgt[tgt_u] = True
        cnt = np.bincount(e_src, minlength=N)
        m1 = (cnt[e_src] == 1) & (~is_tgt[e_src])

        # per window: first KD*128 mult-1 edges are direct
        direct = np.zeros(len(e_src), bool)
        for w in range(W):
            idx = np.nonzero(m1 & (e_win == w))[0]
            assert len(idx) >= KD * P, (c, w, len(idx))
            direct[idx[:KD * P]] = True
        gcnt = np.bincount(e_win[~direct], minlength=W)
        kg_req = max(kg_req, int(np.ceil(gcnt.max() / P)))

        # tabLo rows: targets first, then unique gathered sources
        g_src = e_src[~direct]
        nrow = np.full(N, -1, np.int64)
        nrow[tgt_u] = trow
        extras = np.setdiff1d(np.unique(g_src), tgt_u)
        nrow[extras] = TP + np.arange(len(extras))
        NL_req = max(NL_req, TP + len(extras))
        info.append((tgt_u, inv_k, trow, e_src, e_trow, e_win, direct, nrow))

    KLO = kg_req
    KHI = KD
    EC = KLO + KHI
    cap = EC * P
    ND = W * KD * P                    # direct nodes (8960)
    NDP = ((ND + 1023) // 1024) * 1024  # padded to chunk mult (9216)
    NL = ((NL_req + 1023) // 1024) * 1024
    NPAD = NDP + NL
    B1 = NL  # tabLo tiles first; direct tiles start here

    for c in range(CORES):
        tgt_u, inv_k, trow, e_src, e_trow, e_win, direct, nrow = info[c]
        x_nodes = np.zeros(NPAD, np.int64)

        etcol = np.full((P, W * EC), -1.0, np.float32)
        esrc_g = np.zeros((W, KLO * P), np.int64)

        # direct edges: block b = w*KD + jd -> xTi position NL + b*128 + p
        for w in range(W):
            idx = np.nonzero(direct & (e_win == w))[0]
            for jd in range(KD):
                blk = idx[jd * P:(jd + 1) * P]
                b = w * KD + jd
                x_nodes[NL + b * P:NL + (b + 1) * P] = e_src[blk]
                etcol[:, w * EC + KLO + jd] = \
                    (e_trow[blk] - w * P).astype(np.float32)

        # gathered edges per window, packed into slots [0, KLO*P)
        for w in range(W):
            idx = np.nonzero((~direct) & (e_win == w))[0]
            ng = len(idx)
            esrc_g[w, :ng] = nrow[e_src[idx]]
            ec_ = np.full(KLO * P, -1.0, np.float32)
            ec_[:ng] = (e_trow[idx] - w * P).astype(np.float32)
            etcol[:, w * EC:w * EC + KLO] = \
                ec_.reshape(KLO, P).T

        # tabLo node ids at xTi positions [0, NL)
        rows_used = np.nonzero(nrow >= 0)[0]
        x_nodes[nrow[rows_used]] = rows_used

        # etrow for partition-broadcast (slot-major per window)
        etrow_b = np.empty((1, W * cap), BF16)
        for w in range(W):
            etrow_b[0, w * cap:(w + 1) * cap] = \
                etcol[:, w * EC:(w + 1) * EC].T.reshape(-1).astype(BF16)

        eidx_lo = np.concatenate(
            [_wrap16(esrc_g[w]) for w in range(W)], axis=1)

        kvals = np.zeros(TP, np.int64)
        kvals[:KC] = trow[inv_k]
        kidx = _wrap16(kvals)

        xT = np.zeros((IN, NPAD), BF16)
        xT[:, :] = x[x_nodes].T
        CW = 1024
        assert NPAD % CW == 0
        xTi = np.empty((P, 2 * NPAD), BF16)
        for i in range(NPAD // CW):
            xTi[:, 2 * i * CW:2 * i * CW + CW] = xT[0:P, i * CW:(i + 1) * CW]
            xTi[:, 2 * i * CW + CW:2 * (i + 1) * CW] = \
                xT[P:IN, i * CW:(i + 1) * CW]

        iblob = np.concatenate([eidx_lo, kidx], axis=1)
        per_core.append(dict(xTi=xTi, iblob=iblob,
                             etcol=np.ascontiguousarray(etcol),
                             etrow=etrow_b))
    return per_core, NPAD, EC, KLO, B1


def make_weights(w_in, b_in, w_proj, a_src, a_trg, w_skip):
    w_in = np.asarray(w_in, np.float32)
    b_in = np.asarray(b_in, np.float32)
    w_proj = np.asarray(w_proj, np.float32)
    a_src = np.asarray(a_src, np.float32).reshape(NH, HD)
    a_trg = np.asarray(a_trg, np.float32).reshape(NH, HD)
    w_skip = np.asarray(w_skip, np.float32)

    w1T = np.ascontiguousarray(w_in.T).astype(BF16)        # [256,128]
    b1 = b_in.reshape(H, 1).astype(np.float32)
    # B_src[h, a] = sum_d w_proj[a*16+d, h] * a_src[a, d]
    wp3 = w_proj.reshape(NH, HD, H)
    B_src = np.einsum("adh,ad->ha", wp3, a_src).astype(np.float32)  # [128,8]
    B_trg = np.einsum("adh,ad->ha", wp3, a_trg).astype(BF16)
    w2 = np.zeros((H, 256), np.float32)  # cast to bf16 below
    w2[:, :OC] = w_proj.T
    w2[:, OC:OC + NH] = B_src
    wskT = np.ascontiguousarray(w_skip.T).astype(BF16)     # [128,128]
    iota4 = np.arange(P, dtype=BF16)[None, :].repeat(P, axis=0)
    iota_c = np.arange(P, dtype=np.float32).reshape(P, 1)
    bfblob = np.concatenate(
        [np.ascontiguousarray(w1T[0:P]), np.ascontiguousarray(w1T[P:IN]),
         w2.astype(BF16), wskT, B_trg, iota4], axis=1)  # [128, 776]
    return dict(bfblob=bfblob, b1=b1, iota_c=iota_c)


# ----------------------------------------------------------------------------
# bass kernel
# ----------------------------------------------------------------------------

_BUILD_CACHE = {}


def build(NPAD, EC, KLO, B1):
    key = (NPAD, EC, KLO, B1)
    if key in _BUILD_CACHE:
        return _BUILD_CACHE[key]

    import concourse.bacc as bacc
    import concourse.mybir as mybir
    import concourse.tile as tile

    dt = mybir.dt
    F32 = dt.float32
    F32R = dt.float32r
    I16 = dt.int16
    BF = dt.bfloat16
    AF = mybir.ActivationFunctionType
    OP = mybir.AluOpType

    NT = NPAD // 512
    cap = EC * P
    KHI = EC - KLO

    nc = bacc.Bacc("TRN2", target_bir_lowering=False,
                   num_swdge_queues=4)

    with tile.TileContext(nc) as tc:
        with tc.tile_pool(name="dram", bufs=1, space="DRAM") as dram:
            def din(name, shape, dtp):
                return dram.tile(shape, dtp, kind="ExternalInput", name=name,
                                 uniquify=False)

            NBF = H + H + 256 + OC + NH + P  # 776
            NI16 = W * KLO * 8 + TP // 16
            xTi = din("xTi", [P, 2 * NPAD], BF)
            bfblob = din("bfblob", [P, NBF], BF)
            fblob = din("fblob", [P, 2 + W * EC], F32)
            iblob = din("iblob", [P, NI16], I16)
            etrow = din("etrow", [1, W * cap], BF)

            tabLo = dram.tile([NPAD - B1, 256], BF, kind="Internal",
                              name="tabLo", uniquify=False)
            outT = dram.tile([TP, OC], BF, kind="Internal", name="outT",
                             uniquify=False)
            out = dram.tile([TP, OC], BF, kind="ExternalOutput", name="out",
                            uniquify=False)

        with tc.tile_pool(name="pers", bufs=1) as pers:
            bfb = pers.tile([P, NBF], BF)
            fb = pers.tile([P, 2 + W * EC], F32)
            ib = pers.tile([P, NI16], I16)
            hfmt = pers.tile([H, TP], BF)         # targets' h, feature-major
            strg = pers.tile([P, W * NH], BF)     # per-window s_trg  [t, 8]
            skips = pers.tile([P, W, OC], BF)     # per-window skip   [t, oc]
            st_sb = pers.tile([P, W, EC, NH], BF)   # s_trg per edge slot
            Mw = pers.tile([P, W * cap], BF)      # edge->target one-hot
            Ghi = pers.tile([P, W, EC - KLO, 256], BF)  # hi-gathered rows
            etws = pers.tile([1, W * cap], BF)

            nc.sync.dma_start(etws[:], etrow[:])
            nc.sync.dma_start(fb[:], fblob[:])
            nc.sync.dma_start(bfb[:], bfblob[:])
            nc.sync.dma_start(ib[:], iblob[:])

            w1a = bfb[:, 0:H]
            w1b = bfb[:, H:2 * H]
            w2s = bfb[:, 2 * H:2 * H + 256]
            wsks = bfb[:, 2 * H + 256:2 * H + 256 + OC]
            btrgs = bfb[:, 2 * H + 256 + OC:2 * H + 256 + OC + NH]
            iota4s = bfb[:, 2 * H + 256 + OC + NH:NBF]
            b1s = fb[:, 0:1]
            iotac = fb[:, 1:2]
            etcols = fb[:, 2:2 + W * EC]
            eloidx = ib[:, 0:W * KLO * 8]
            kidxs = ib[:, W * KLO * 8:NI16]

            CH = 2  # 512-node tiles per xT load chunk
            with tc.tile_pool(name="pa", bufs=2) as pa, \
                 tc.tile_pool(name="pax", bufs=3) as pax, \
                 tc.tile_pool(name="pbc", bufs=2) as pbc, \
                 tc.tile_pool(name="pmtw", bufs=1) as pmtw, \
                 tc.tile_pool(name="pghi", bufs=3) as pghi, \
                 tc.tile_pool(name="pe2", bufs=2) as pe2, \
                 tc.tile_pool(name="psa", bufs=2, space="PSUM") as psa, \
                 tc.tile_pool(name="psb", bufs=2, space="PSUM") as psb, \
                 tc.tile_pool(name="psc", bufs=1, space="PSUM") as psc, \
                 tc.tile_pool(name="psd", bufs=1, space="PSUM") as psd, \
                 tc.tile_pool(name="pse", bufs=2, space="PSUM") as pse:

                # ---- partition-broadcast of per-slot target cols (Pool) ----
                pbcs = []
                for w in range(W):
                    pbcw = pbc.tile([P, cap], BF, tag="pbcw")
                    nc.gpsimd.partition_broadcast(
                        pbcw[:], etws[0:1, w * cap:(w + 1) * cap])
                    pbcs.append(pbcw)

                # deferred emissions interleaved into phase A slack
                mtws = {}

                def emit_mtw(w):
                    Mtw = pmtw.tile([P, cap], BF, tag="Mtw")
                    nc.vector.tensor_scalar(Mtw[:], pbcs[w][:], iotac[:], None,
                                            OP.is_equal)
                    mtws[w] = Mtw

                def emit_loop1(w):
                    # s_trg / skip for the window targets
                    stp = psd.tile([P, OC], F32, tag="misc")
                    nc.tensor.matmul(stp[:, 0:NH],
                                     lhsT=hfmt[:, w * P:(w + 1) * P],
                                     rhs=btrgs[:], start=True, stop=True)
                    nc.vector.tensor_copy(strg[:, w * NH:(w + 1) * NH],
                                            stp[:, 0:NH])
                    skp = psd.tile([P, OC], F32, tag="misc")
                    nc.tensor.matmul(skp[:], lhsT=hfmt[:, w * P:(w + 1) * P],
                                     rhs=wsks[:], start=True, stop=True)
                    nc.vector.tensor_copy(skips[:, w], skp[:])
                    # s_trg edge-slot expansion via the col-major one-hot
                    Mtw = mtws.pop(w)
                    stps = psc.tile([P, EC, NH], F32, tag="stps")
                    for j in range(EC):
                        nc.tensor.matmul(
                            stps[:, j, :], lhsT=Mtw[:, j * P:(j + 1) * P],
                            rhs=strg[:, w * NH:(w + 1) * NH],
                            start=True, stop=True)
                    nc.vector.tensor_copy(st_sb[:, w], stps[:])

                def emit_mw(w, j, eng=None):
                    col = w * EC + j
                    (eng or nc.vector).tensor_scalar(
                        Mw[:, col * P:(col + 1) * P], iota4s[:, 0:P],
                        etcols[:, col:col + 1], None, OP.is_equal)

                # schedule: loop1(w) at tile 2+w; Mw slots spread over tiles

                # ------- phase A (tabLo tiles first, then direct) -------
                LT = B1 // 512  # tabLo tiles
                for ci in range(NT // CH):
                    t0 = ci * CH
                    wdc = CH * 512
                    xc = pax.tile([P, 2 * wdc], BF, tag="xc")
                    nc.sync.dma_start(xc[:], xTi[:, 2 * ci * wdc:
                                                 2 * (ci + 1) * wdc])
                    stg = pa.tile([P, 2, 4, 256], BF, tag="stg")
                    for t in range(t0, t0 + CH):
                        o = (t - t0) * 512
                        hps = psa.tile([P, 512], F32, tag="hps")
                        nc.tensor.matmul(hps[:], lhsT=w1a[:],
                                         rhs=xc[:, o:o + 512],
                                         start=True, stop=False)
                        nc.tensor.matmul(hps[:], lhsT=w1b[:],
                                         rhs=xc[:, wdc + o:wdc + o + 512],
                                         start=False, stop=True)
                        hsb = pa.tile([P, 512], BF, tag="hsb")
                        nc.scalar.activation(hsb[:], hps[:], AF.Relu,
                                             bias=b1s[:])
                        if t * 512 < TP:
                            w0 = t * 512
                            w1_ = min(TP, w0 + 512)
                            nc.scalar.activation(hfmt[:, w0:w1_],
                                                 hps[:, 0:(w1_ - w0)], AF.Relu,
                                                 bias=b1s[:])
                        for half in range(2):
                            p2 = psb.tile([P, 2, 256], F32, tag="p2")
                            for jj in range(2):
                                j = half * 2 + jj
                                nc.tensor.matmul(
                                    p2[:, jj, :],
                                    lhsT=hsb[:, j * P:(j + 1) * P],
                                    rhs=w2s[:], start=True, stop=True)
                            if t >= LT:
                                # direct class: copy straight into Ghi; the
                                # host packed these nodes in edge-slot order
                                b0 = (t - LT) * 4 + half * 2
                                w_, jd = b0 // KHI, b0 % KHI
                                if jd != KHI - 1 and b0 + 1 < W * KHI:
                                    # both blocks same window, adjacent slots
                                    dst = Ghi[:, w_, jd:jd + 2, 0:OC + NH]
                                    s2 = p2[:, :, 0:OC + NH]
                                    if half == 0:
                                        nc.scalar.activation(dst, s2, AF.Copy)
                                    else:
                                        nc.vector.tensor_copy(dst, s2)
                                else:
                                    for sub in range(2):
                                        b = b0 + sub
                                        if b >= W * KHI:
                                            continue
                                        wx, jx = b // KHI, b % KHI
                                        dst = Ghi[:, wx, jx, 0:OC + NH]
                                        s1 = p2[:, sub, 0:OC + NH]
                                        if half == 0:
                                            nc.scalar.activation(dst, s1,
                                                                 AF.Copy)
                                        else:
                                            nc.vector.tensor_copy(dst, s1)
                            else:
                                sgh = stg[:, t - t0, half * 2:half * 2 + 2, :]
                                if half == 0:
                                    nc.scalar.activation(sgh[:, :, 0:OC + NH],
                                                         p2[:, :, 0:OC + NH],
                                                         AF.Copy)
                                else:
                                    nc.vector.tensor_copy(sgh[:, :, 0:OC + NH],
                                                          p2[:, :, 0:OC + NH])
                    if t0 < LT:
                        rr = t0 * 512
                        nc.sync.dma_start(
                            tabLo[rr:rr + CH * 512, :].rearrange(
                                "(i j p) f -> p i j f", p=P, i=CH), stg[:])
                    # interleaved loop-1 / mask emissions (by position)
                    for pi in (ci * CH, ci * CH + 1):
                        if 1 <= pi <= 2 * W and pi % 2 == 1:
                            emit_mtw((pi - 1) // 2)
                        if 2 <= pi <= 2 * W + 1 and pi % 2 == 0:
                            emit_loop1((pi - 2) // 2)

                # edge->target one-hot masks: fills the DVE gap between
                # phase A and the window chains; every 4th slot on Pool
                for w_ in range(W):
                    for j_ in range(EC):
                        emit_mw(w_, j_)

                # ---------------- gathers ----------------
                # direct-class rows are already in Ghi (phase A); only the
                # gathered class reads tabLo, in window pairs
                glos = []
                for w in range(W):
                    G = pghi.tile([P, KLO, 256], BF, tag="G")
                    nc.gpsimd.dma_gather(
                        G[:], tabLo[:],
                        eloidx[:, w * KLO * 8:(w + 1) * KLO * 8],
                        KLO * P, KLO * P, 256, single_packet=False,
                        queue_num=1 + w % 3)
                    glos.append(G)

                # ---------------- loop 2: per-window edge pipeline ----------
                def finalize(w, segp):
                    den = pe2.tile([P, NH], F32, tag="den")
                    nc.vector.tensor_scalar_add(den[:], segp[:, OC:OC + NH],
                                                EPS)
                    rec = pe2.tile([P, NH], F32, tag="rec")
                    nc.vector.reciprocal(rec[:], den[:])
                    z = pe2.tile([P, OC], F32, tag="z")
                    recb = rec[:].broadcast_to([P, NH, HD])
                    nc.vector.tensor_tensor(
                        z[:].rearrange("p (a d) -> p a d", d=HD),
                        segp[:, 0:OC].rearrange("p (a d) -> p a d", d=HD),
                        recb, OP.mult)
                    nc.gpsimd.tensor_add(z[:], z[:], skips[:, w])
                    # elu: (max(z,0)-1) + exp(min(z,0))
                    am = pe2.tile([P, OC], BF, tag="am")
                    nc.gpsimd.tensor_scalar(am[:], z[:], 0.0, -1.0, OP.max,
                                            OP.add)
                    bm = pe2.tile([P, OC], BF, tag="bm")
                    nc.gpsimd.tensor_scalar(bm[:], z[:], 0.0, None, OP.min)
                    eb = pe2.tile([P, OC], BF, tag="eb")
                    nc.scalar.activation(eb[:], bm[:], AF.Exp)
                    nc.vector.tensor_add(am[:], am[:], eb[:])
                    nc.sync.dma_start(outT[w * P:(w + 1) * P, :], am[:])

                # software-pipelined window stages: each engine's
                # in-order queue interleaves adjacent windows
                st1 = {}   # w -> (sc-dependent) emax tile
                st2 = {}   # w -> Wv tile
                st3 = {}   # w -> segp psum tile

                def stage1(w):
                    G = glos[w]
                    sc = pe2.tile([P, EC, NH], F32, tag="sc")
                    nc.vector.tensor_tensor(sc[:, 0:KLO], st_sb[:, w, 0:KLO],
                                            G[:, :, OC:OC + NH], OP.add)
                    nc.vector.tensor_tensor(sc[:, KLO:EC],
                                            st_sb[:, w, KLO:EC],
                                            Ghi[:, w, :, OC:OC + NH], OP.add)
                    # exp(lrelu(s)) = exp(max(s, 0.2*s)): max first, 1 exp
                    e2 = pe2.tile([P, EC, NH], BF, tag="e2")
                    nc.vector.tensor_scalar(e2[:], sc[:], 0.2, None, OP.mult)
                    nc.vector.tensor_max(e2[:], e2[:], sc[:])
                    e1 = pe2.tile([P, EC, NH], BF, tag="e1")
                    nc.scalar.activation(e1[:], e2[:], AF.Exp)
                    st1[w] = e1

                def stage2(w):
                    G = glos[w]
                    emax = st1.pop(w)
                    Wv = pe2.tile([P, EC, 136], BF, tag="Wv")
                    nc.vector.tensor_copy(Wv[:, :, OC:OC + NH], emax[:])
                    emb = emax[:].broadcast_to([P, EC, NH, HD])
                    pool = pmtw if w % 2 == 0 else pbc
                    tag = "Mtw" if w % 2 == 0 else "pbcw"
                    eex = pool.tile([P, cap], BF, tag=tag)
                    ex3 = eex[:].rearrange("p (j f) -> p j f", f=P)
                    nc.scalar.activation(
                        ex3.rearrange("p j (a d) -> p j a d", d=HD),
                        emb, AF.Copy)
                    nc.vector.tensor_tensor(Wv[:, 0:KLO, 0:OC],
                                            G[:, :, 0:OC],
                                            ex3[:, 0:KLO], OP.mult)
                    nc.vector.tensor_tensor(Wv[:, KLO:EC, 0:OC],
                                            Ghi[:, w, :, 0:OC],
                                            ex3[:, KLO:EC], OP.mult)
                    st2[w] = Wv

                def stage3(w):
                    Wv = st2.pop(w)
                    segp = pse.tile([P, 136], F32, tag="segp")
                    for j in range(EC):
                        nc.tensor.matmul(segp[:],
                                         lhsT=Mw[:, (w * EC + j) * P:
                                                 (w * EC + j + 1) * P],
                                         rhs=Wv[:, j, :], start=(j == 0),
                                         stop=(j == EC - 1))
                    st3[w] = segp

                def finalize(w):
                    segp = st3.pop(w)
                    den = pe2.tile([P, NH], F32, tag="den")
                    nc.vector.tensor_scalar_add(den[:], segp[:, OC:OC + NH],
                                                EPS)
                    rec = pe2.tile([P, NH], F32, tag="rec")
                    nc.vector.reciprocal(rec[:], den[:])
                    z = pe2.tile([P, OC], F32, tag="z")
                    recb = rec[:].broadcast_to([P, NH, HD])
                    nc.vector.tensor_tensor(
                        z[:].rearrange("p (a d) -> p a d", d=HD),
                        segp[:, 0:OC].rearrange("p (a d) -> p a d", d=HD),
                        recb, OP.mult)
                    nc.gpsimd.tensor_add(z[:], z[:], skips[:, w])
                    # elu: (max(z,0)-1) + exp(min(z,0))
                    am = pe2.tile([P, OC], BF, tag="am")
                    nc.gpsimd.tensor_scalar(am[:], z[:], 0.0, -1.0, OP.max,
                                            OP.add)
                    bm = pe2.tile([P, OC], BF, tag="bm")
                    nc.gpsimd.tensor_scalar(bm[:], z[:], 0.0, None, OP.min)
                    eb = pe2.tile([P, OC], BF, tag="eb")
                    nc.scalar.activation(eb[:], bm[:], AF.Exp)
                    nc.vector.tensor_add(am[:], am[:], eb[:])
                    nc.sync.dma_start(outT[w * P:(w + 1) * P, :], am[:])

                for w in range(W + 3):
                    if w < W:
                        stage1(w)
                    if 1 <= w <= W:
                        stage2(w - 1)
                    if 2 <= w <= W + 1:
                        stage3(w - 2)
                    if 3 <= w:
                        finalize(w - 3)

                # final k-row gather (reuses a pghi slot)
                kob = pghi.tile([P, KLO, 256], BF, tag="G")
                ko = kob[:].rearrange("p k f -> p (k f)")[:, 0:TP // P * OC]
                ko3 = ko.rearrange("p (j f) -> p j f", f=OC)
                nc.gpsimd.dma_gather(ko3, outT[:], kidxs[:], TP, TP, OC,
                                     single_packet=False)
                nc.sync.dma_start(
                    out[:].rearrange("(j p) f -> p j f", p=P), ko3)

    nc.compile()
    _BUILD_CACHE[key] = nc
    return nc


# ----------------------------------------------------------------------------
# entry point
# ----------------------------------------------------------------------------

def kernel(x, adj0, index0, w_in, b_in, w_proj, a_src, a_trg, w_skip):
    from concourse.bass_utils import run_bass_kernel_spmd

    per_core, NPAD, EC, KLO, B1 = plan(x, adj0, index0)
    wts = make_weights(w_in, b_in, w_proj, a_src, a_trg, w_skip)
    nc = build(NPAD, EC, KLO, B1)

    in_maps = []
    for c in range(CORES):
        pc = per_core[c]
        fblob = np.concatenate(
            [wts["b1"], wts["iota_c"], pc["etcol"]], axis=1).astype(np.float32)
        in_maps.append(dict(bfblob=wts["bfblob"], fblob=fblob,
                            xTi=pc["xTi"], iblob=pc["iblob"],
                            etrow=pc["etrow"]))

    res = run_bass_kernel_spmd(nc, in_maps, core_ids=list(range(CORES)))
    outs = [r["out"][:KC] for r in res.results]
    return np.concatenate(outs, axis=0).astype(np.float32)
